# revision 22
# baseline (speedup 1.0000x reference)
"""DualRoadGNN Trainium2 kernel: 8-core SPMD, sharded by graph.

Layout: feature-major per graph ([H partitions, node columns]); graphs padded
500 -> 512 nodes. GCN message passing runs as dense matmuls against per-graph
adjacency matrices built on device from host-shipped integer edge lists
(local_scatter of host-prescaled coefficients cnt*dinv[s]*dinv[d]).
KNN road: cosine sim via PE matmul (bf16 inputs, f32 accum), top-k via DVE
max/max_index reading PSUM directly, adjacency via local_scatter + DMA-xbar
transpose. GraphNorm stats via DVE bn_stats on PSUM; the affine+prelu is a
single ACT op reading PSUM with the conv bias absorbed. Small per-graph
scalar math runs on the Pool engine to keep DVE off the critical path.
"""
import contextlib
import os
import sys

sys.path.insert(0, "/opt/trn_rl_repo")
import numpy as np

import concourse.bacc as bacc
import concourse.tile as tile
from concourse import mybir
from concourse.bass_utils import run_bass_kernel_spmd

G, NPG, NP = 100, 500, 512
IN, H, L = 128, 256, 2   # L = executed layer iterations (range(3-1) in the model)
W = 64                   # max unique out-edges per source node (incl self loop)
N_CORES = 8
GPC = 13                 # graph slots per core
STARTS = [0, 13, 26, 39, 52, 64, 76, 88, 100]
NGS = [STARTS[i + 1] - STARTS[i] for i in range(N_CORES)]
F32 = mybir.dt.float32
BF16 = mybir.dt.bfloat16

# fvec column map
FV_EMB_B = 0
FV_GATE_B = 2
FV_L = 4   # then per layer: conv_b, norm_w, norm_b, norm_ms, fconv_b, fnorm_w, fnorm_b, fnorm_ms
FV_N = 4 + L * 16


def build_program(gpc):
    nc = bacc.Bacc("TRN2", target_bir_lowering=False, debug=False, num_devices=N_CORES)
    d = {}
    d["xT"] = nc.dram_tensor("xT", [gpc, IN, NP], BF16, kind="ExternalInput")
    d["ei"] = nc.dram_tensor("ei", [gpc, 4, 128, W], mybir.dt.int16, kind="ExternalInput")
    d["ev"] = nc.dram_tensor("ev", [gpc, 4, 128, W], BF16, kind="ExternalInput")
    d["embW"] = nc.dram_tensor("embW", [IN, H], BF16, kind="ExternalInput")
    d["convW"] = nc.dram_tensor("convW", [L, H, H], BF16, kind="ExternalInput")
    d["fconvW"] = nc.dram_tensor("fconvW", [L, H, H], BF16, kind="ExternalInput")
    d["gateW"] = nc.dram_tensor("gateW", [2 * H, H], BF16, kind="ExternalInput")
    d["fvec"] = nc.dram_tensor("fvec", [128, FV_N], F32, kind="ExternalInput")
    d["gf"] = nc.dram_tensor("gf", [gpc, H], F32, kind="ExternalOutput")

    with tile.TileContext(nc) as tc:
        _emit(nc, tc, gpc, d)
    nc.compile()
    return nc


def _emit(nc, tc, gpc, d):
    AF = mybir.ActivationFunctionType
    OP = mybir.AluOpType
    I32 = mybir.dt.int32
    import concourse.bass as bass

    ctx = contextlib.ExitStack()
    with ctx:
        sg = ctx.enter_context(tc.tile_pool(name="singles", bufs=1))
        pg = ctx.enter_context(tc.tile_pool(name="pg", bufs=3))
        dp = ctx.enter_context(tc.tile_pool(name="dramp", bufs=1, space="DRAM"))
        psC = ctx.enter_context(tc.tile_pool(name="psC", bufs=3, space="PSUM"))
        psS = ctx.enter_context(tc.tile_pool(name="psS", bufs=2, space="PSUM"))
        psM = ctx.enter_context(tc.tile_pool(name="psM", bufs=2, space="PSUM"))

        def T(shape, dtype=F32, tag=None, pool=pg, bufs=None):
            kw = {} if bufs is None else {"bufs": bufs}
            return pool.tile(shape, dtype, name=tag, tag=tag, **kw)

        # Pool has no TensorScalarPtr; scalar constants live in small SBUF tiles
        consts = {}

        def newton_rsqrt(eng, v_ap, out_tile, tmp_tile, iters):
            """out = 1/sqrt(v): bit-trick seed on DVE (Pool lacks i32 shift),
            Newton polish on `eng` (tensor_tensor only)."""
            n = v_ap.shape[1]
            y = out_tile
            nc.vector.tensor_scalar(out=y.bitcast(I32), in0=v_ap.bitcast(I32), scalar1=1, scalar2=None,
                                    op0=OP.arith_shift_right)
            nc.vector.tensor_scalar(out=y.bitcast(I32), in0=y.bitcast(I32), scalar1=-1, scalar2=0x5F3759DF,
                                    op0=OP.mult, op1=OP.add)
            for it in range(iters):
                eng.tensor_mul(tmp_tile, y, y)
                eng.tensor_mul(tmp_tile, tmp_tile, v_ap)
                eng.tensor_mul(tmp_tile, tmp_tile, consts["half"][:, 0:n])
                eng.tensor_tensor(out=tmp_tile, in0=consts["c15"][:, 0:n], in1=tmp_tile, op=OP.subtract)
                eng.tensor_mul(y, y, tmp_tile)
            return y

        # --- resident constants/weights ---
        embW = T([128, H], BF16, tag="embW_t", pool=sg)
        nc.sync.dma_start(out=embW, in_=d["embW"][:, :])
        convW = {}
        for l in range(L):
            for k in range(2):
                t = T([128, H], BF16, tag=f"convW{l}_{k}", pool=sg)
                nc.sync.dma_start(out=t, in_=d["convW"][l, k * 128:(k + 1) * 128, :])
                convW[(l, k)] = t
                t2 = T([128, H], BF16, tag=f"fconvW{l}_{k}", pool=sg)
                nc.sync.dma_start(out=t2, in_=d["fconvW"][l, k * 128:(k + 1) * 128, :])
                convW[(l, k, "f")] = t2
        gateW = []
        for c in range(4):
            t = T([128, H], BF16, tag=f"gateW{c}", pool=sg)
            nc.sync.dma_start(out=t, in_=d["gateW"][c * 128:(c + 1) * 128, :])
            gateW.append(t)
        fvec = T([128, FV_N], tag="fvec_t", pool=sg)
        nc.sync.dma_start(out=fvec, in_=d["fvec"][:, :])

        onesb = T([128, 1], BF16, tag="onesb", pool=sg)
        nc.vector.memset(onesb, 1.0)
        for nm, dt_, val in [("shift1", I32, 1), ("magic", I32, 0x5F3759DF),
                             ("half", F32, 0.5), ("c15", F32, 1.5),
                             ("eps", F32, 1e-5), ("cinv", F32, 1.0 / NPG)]:
            t = T([128, 4], dt_, tag=f"const_{nm}", pool=sg)
            nc.vector.memset(t, val)
            consts[nm] = t
        q4 = T([128, 4], BF16, tag="q4", pool=sg)
        nc.vector.memset(q4, 0.25)
        nc.vector.memset(q4[:, 0:1], 0.5)
        nc.vector.memset(q4[:, 3:4], 0.0)

        def fv(col, n=1):
            return fvec[:, col:col + n]

        def road(inT, Wk0, Wk1, Amat, b_col, nw_col, nb_col, nms_col, otag):
            # m = h @ W in node-major chunks
            m = []
            for sc in range(4):
                ps = psM.tile([128, H], F32, name="psm", tag="psm", bufs=2)
                nc.tensor.matmul(ps, lhsT=inT[0][:, sc * 128:(sc + 1) * 128], rhs=Wk0, start=True, stop=False)
                nc.tensor.matmul(ps, lhsT=inT[1][:, sc * 128:(sc + 1) * 128], rhs=Wk1, start=False, stop=True)
                mt = T([128, H], BF16, tag=f"m_{sc}", bufs=4)
                nc.scalar.copy(mt, ps)
                m.append(mt)
            # c = A^T m, kept in PSUM (no bias; absorbed into the prelu affine)
            cps = []
            for k in range(2):
                ps = psC.tile([128, NP], F32, name="psc", tag="psc", bufs=3)
                for sc in range(4):
                    nc.tensor.matmul(ps, lhsT=m[sc][:, k * 128:(k + 1) * 128], rhs=Amat[:, sc, :],
                                     start=(sc == 0), stop=(sc == 3))
                cps.append(ps)
            # GraphNorm stats straight off PSUM
            mv4 = T([128, 4], tag="mv4", bufs=4)
            for k in range(2):
                stats = T([128, 6], tag="bnst", bufs=4)
                nc.vector.bn_stats(out=stats, in_=cps[k][:, 0:NPG])
                nc.vector.bn_aggr(out=mv4[:, 2 * k:2 * k + 2], in_=stats)
            mvv = mv4.rearrange("p (a b) -> p a b", b=2)
            m2 = mvv[:, :, 0]   # mean of ps (pre-bias)
            v2 = mvv[:, :, 1]   # var (shift-invariant)
            cb = fv(b_col, 2)
            # small-op chain on DVE (single engine: serial chains on one
            # in-order queue don't block other engines' streams)
            mc = T([128, 2], tag="mc", bufs=4)
            nc.vector.tensor_tensor(out=mc, in0=m2, in1=cb, op=OP.add)
            tms = T([128, 2], tag="tms", bufs=4)
            nc.vector.tensor_tensor(out=tms, in0=mc, in1=fv(nms_col, 2), op=OP.mult)
            tb = T([128, 2], tag="tb", bufs=4)
            nc.vector.tensor_tensor(out=tb, in0=mc, in1=tms, op=OP.subtract)
            tb2 = T([128, 2], tag="tb2", bufs=4)
            nc.vector.tensor_mul(tb2, tb, tb)
            u2 = T([128, 2], tag="u2", bufs=4)
            nc.vector.scalar_tensor_tensor(out=u2, in0=tb2, scalar=1e-5, in1=v2, op0=OP.add, op1=OP.add)
            rstd2 = T([128, 2], tag="rstd2", bufs=4)
            ntmp2 = T([128, 2], tag="ntmp2", bufs=4)
            newton_rsqrt(nc.vector, u2, rstd2, ntmp2, 1)
            wr2 = T([128, 2], tag="wr2", bufs=4)
            nc.vector.tensor_tensor(out=wr2, in0=rstd2, in1=fv(nw_col, 2), op=OP.mult)
            t2c = T([128, 2], tag="t2c", bufs=4)
            nc.vector.tensor_tensor(out=t2c, in0=cb, in1=tms, op=OP.subtract)
            t3c = T([128, 2], tag="t3c", bufs=4)
            nc.vector.tensor_mul(t3c, wr2, t2c)
            bb2 = T([128, 2], tag="bb2", bufs=4)
            nc.vector.tensor_tensor(out=bb2, in0=fv(nb_col, 2), in1=t3c, op=OP.add)
            outT = []
            for k in range(2):
                oT = T([128, NP], BF16, tag=f"{otag}_{k}", bufs=4)
                nc.scalar.activation(out=oT, in_=cps[k], func=AF.Prelu, bias=bb2[:, k:k + 1],
                                     scale=wr2[:, k:k + 1], alpha=0.01)
                outT.append(oT)
            return outT

        def frontA(i):
            st = {}
            xT = T([128, NP], BF16, tag="xT_t", bufs=2)
            nc.sync.dma_start(out=xT, in_=d["xT"][i])
            eit = T([128, 4, W], mybir.dt.int16, tag="eit", bufs=2)
            evb = T([128, 4, W], BF16, tag="evb", bufs=2)
            for c in range(4):
                nc.sync.dma_start(out=eit[:, c, :], in_=d["ei"][i, c])
                nc.sync.dma_start(out=evb[:, c, :], in_=d["ev"][i, c])

            # adjacency directly from prescaled coefficients
            AT = T([128, 4, NP], BF16, tag="AT", bufs=8)
            for c in range(4):
                nc.gpsimd.local_scatter(out_ap=AT[:, c, :], data_ap=evb[:, c, :], idxs_ap=eit[:, c, :],
                                        channels=128, num_elems=NP, num_idxs=W)

            # embedding
            hT = []
            for k in range(2):
                ps = psC.tile([128, NP], F32, name="psc", tag="psc", bufs=3)
                nc.tensor.matmul(ps, lhsT=embW[:, k * 128:(k + 1) * 128], rhs=xT, start=True, stop=True)
                t = T([128, NP], BF16, tag=f"hT_{k}", bufs=8)
                nc.scalar.activation(out=t, in_=ps, func=AF.Identity, bias=fv(FV_EMB_B + k))
                hT.append(t)

            # row norms: sum of squares via PE, rsqrt on [128,4] via DRAM relayout
            sq = []
            for k in range(2):
                t = T([128, NP], BF16, tag="sq", bufs=2)
                nc.scalar.square(t, hT[k])
                sq.append(t)
            psq = psS.tile([128, 4], F32, name="psq", tag="psq", bufs=1)
            for c in range(4):
                nc.tensor.matmul(psq[:, c:c + 1], lhsT=sq[0][:, c * 128:(c + 1) * 128], rhs=onesb,
                                 start=True, stop=False)
                nc.tensor.matmul(psq[:, c:c + 1], lhsT=sq[1][:, c * 128:(c + 1) * 128], rhs=onesb,
                                 start=False, stop=True)
            rsq4 = T([128, 4], tag="rsq4", bufs=2)
            nc.vector.tensor_copy(out=rsq4, in_=psq)
            st.update(i=i, hT=hT, AT=AT, rsq4=rsq4)
            return st

        def P1(st):  # rsqrt (one iteration after frontA: deps satisfied)
            rin4 = T([128, 4], tag="rin4", bufs=2)
            rtmp4 = T([128, 4], tag="rtmp4", bufs=2)
            newton_rsqrt(nc.vector, st["rsq4"], rin4, rtmp4, 2)
            rb4 = T([128, 4], BF16, tag="rb4", bufs=2)
            nc.vector.tensor_copy(out=rb4, in_=rin4)
            st["rb4"] = rb4

        def P2a(st):  # park rsqrt row to DRAM in node order
            rrowb = dp.tile([1, NP], BF16, name="rrowb", tag="rrowb", bufs=2)
            wr_ap = bass.AP(tensor=rrowb.tensor, offset=rrowb.offset, ap=[[1, 128], [128, 4]])
            nc.sync.dma_start(out=wr_ap, in_=st["rb4"])
            st["rrowb"] = rrowb

        def P2b(st):  # broadcast-read across all partitions
            rrowb = st["rrowb"]
            rb = T([128, NP], BF16, tag="rb", bufs=3)
            rb_ap = bass.AP(tensor=rrowb.tensor, offset=rrowb.offset, ap=[[0, 128], [1, NP]])
            nc.sync.dma_start(out=rb, in_=rb_ap)
            st["rb"] = rb

        def P3(st):  # normalize (deps one iteration old)
            hnT = []
            for k in range(2):
                t = T([128, NP], BF16, tag=f"hnT_{k}", bufs=3)
                nc.vector.tensor_mul(t, st["hT"][k], st["rb"])
                hnT.append(t)
            st["hnT"] = hnT

        def frontB(st):
            hnT = st["hnT"]
            afb = []
            for j in range(4):
                ps = psS.tile([128, NP], F32, name="pss", tag="pss", bufs=2)
                nc.tensor.matmul(ps, lhsT=hnT[0][:, j * 128:(j + 1) * 128], rhs=hnT[0], start=True, stop=False)
                nc.tensor.matmul(ps, lhsT=hnT[1][:, j * 128:(j + 1) * 128], rhs=hnT[1], start=False, stop=True)
                nc.vector.memset(ps[:, NPG:NP], -1e30)
                mx = T([128, 8], tag="mx")
                mi = T([128, 8], mybir.dt.uint16, tag="mi")
                nc.vector.max(mx, ps[:, :])
                nc.vector.max_index(mi, mx, ps[:, :])
                if j == 3:
                    idx4 = T([128, 4], mybir.dt.int16, tag="idx4")
                    nc.vector.memset(idx4, -1)
                    nc.vector.tensor_copy(out=idx4[0:NPG - 384, 0:3], in_=mi[0:NPG - 384, 0:3].bitcast(mybir.dt.int16))
                    scat_idx = idx4[:, :]
                else:
                    scat_idx = mi[:, 0:4].bitcast(mybir.dt.int16)
                af = T([128, NP], BF16, tag=f"afb_{j}", bufs=3)
                nc.gpsimd.local_scatter(out_ap=af[:, :], data_ap=q4[:, :], idxs_ap=scat_idx,
                                        channels=128, num_elems=NP, num_idxs=4)
                afb.append(af)
            st["afb"] = afb

        def gate_update(st, l, h1, h2):
            prevT = st["prevT"]
            newT = []
            for k in range(2):
                ps = psC.tile([128, NP], F32, name="psc", tag="psc", bufs=3)
                for c in range(4):
                    rhs = h1[c] if c < 2 else h2[c - 2]
                    nc.tensor.matmul(ps, lhsT=gateW[c][:, k * 128:(k + 1) * 128], rhs=rhs,
                                     start=(c == 0), stop=(c == 3))
                gT = T([128, NP], BF16, tag="gT", bufs=4)
                nc.scalar.activation(out=gT, in_=ps, func=AF.Sigmoid, bias=fv(FV_GATE_B + k))
                dT = T([128, NP], BF16, tag="dT", bufs=4)
                nc.gpsimd.tensor_sub(dT, h1[k], h2[k])
                t2 = T([128, NP], BF16, tag="t2", bufs=4)
                nc.vector.tensor_mul(t2, gT, dT)
                nc.vector.tensor_add(t2, t2, h2[k])
                hn = T([128, NP], BF16, tag=f"hn{l}_{k}", bufs=4)
                cols = NP if l == 0 else NPG
                nc.vector.tensor_add(hn[:, 0:cols], t2[:, 0:cols], prevT[k][:, 0:cols])
                newT.append(hn)
            st["prevT"] = newT
            return newT

        def S3(st):  # knn adjacency transpose (DMA xbar) + layer0 road1
            afb = st["afb"]
            AfT = T([128, 4, NP], BF16, tag="AfT", bufs=5)
            for j in range(4):
                nc.sync.dma_start_transpose(out=AfT[:, :, j * 128:(j + 1) * 128], in_=afb[j][:, :])
            st["AfT"] = AfT
            st["prevT"] = st["hT"]
            base = FV_L
            st["h1l0"] = road(st["hT"], convW[(0, 0)], convW[(0, 1)], st["AT"],
                              base + 0, base + 2, base + 4, base + 6, "h1l0")

        def S4(st):  # layer0 road2
            base = FV_L
            st["h2l0"] = road(st["h1l0"], convW[(0, 0, "f")], convW[(0, 1, "f")], st["AfT"],
                              base + 8, base + 10, base + 12, base + 14, "h2l0")

        def S5(st):  # gate0 + update0 + layer1 road1
            st["all0"] = gate_update(st, 0, st["h1l0"], st["h2l0"])
            base = FV_L + 16
            st["h1l1"] = road(st["all0"], convW[(1, 0)], convW[(1, 1)], st["AT"],
                              base + 0, base + 2, base + 4, base + 6, "h1l1")

        def S6(st):  # layer1 road2
            base = FV_L + 16
            st["h2l1"] = road(st["h1l1"], convW[(1, 0, "f")], convW[(1, 1, "f")], st["AfT"],
                              base + 8, base + 10, base + 12, base + 14, "h2l1")

        def S7(st):  # gate1 + update1 + pooling sums (ACT accumulate)
            curT = gate_update(st, 1, st["h1l1"], st["h2l1"])
            all0 = st["all0"]
            r0 = T([128, 2], tag="r0", bufs=4)
            r1 = T([128, 2], tag="r1", bufs=4)
            dump = T([128, NPG], BF16, tag="rdump", bufs=2)
            for k in range(2):
                nc.scalar.activation(out=dump, in_=all0[k][:, 0:NPG], func=AF.Identity,
                                     accum_out=r0[:, k:k + 1])
                nc.scalar.activation(out=dump, in_=curT[k][:, 0:NPG], func=AF.Identity,
                                     accum_out=r1[:, k:k + 1])
            st["r0"] = r0
            st["r1"] = r1

        def S8(st):  # combine pooled sums (one iteration later) + output
            i = st["i"]
            r0 = st["r0"]
            r1 = st["r1"]
            gfo = T([128, 2], tag="gfo")
            nc.gpsimd.tensor_add(gfo, r1, r1)
            nc.gpsimd.tensor_add(gfo, gfo, r0)
            nc.gpsimd.tensor_mul(gfo, gfo, consts["cinv"][:, 0:2])
            nc.sync.dma_start(out=d["gf"][i].rearrange("(k p) -> p k", p=128), in_=gfo)

        # software pipeline across graphs: every stage's inputs were produced
        # at least one iteration earlier, so each engine's in-order stream
        # almost never waits within an iteration. The rsqrt/broadcast chain
        # is spread over P1/P2a/P2b/P3 so no queue blocks on a same-iteration
        # cross-engine dependency.
        stages = [S3, S4, S5, S6, S7, S8]
        tails = [P1, P2a, P2b]
        window = []
        for i in range(gpc + 11):
            if i < gpc:
                st = frontA(i)
                window.append(st)
            for si, fn in enumerate(stages):
                gi = i - 5 - si
                if 0 <= gi < gpc:
                    fn(window[gi])
            for si, fn in enumerate(tails):
                gi = i - 1 - si
                if 0 <= gi < gpc:
                    fn(window[gi])
            if 0 <= i - 4 < gpc:
                P3(window[i - 4])
                frontB(window[i - 4])


def prep_inputs(inputs):
    """Build the 8 per-core input maps from full-problem inputs."""
    x = np.asarray(inputs["x"], np.float32)
    edge_index = np.asarray(inputs["edge_index"], np.int64)
    batch = np.asarray(inputs["batch"], np.int64)
    N = G * NPG
    assert x.shape == (N, IN)
    assert np.array_equal(batch, np.repeat(np.arange(G), NPG)), "non-uniform batch unsupported"

    src, dst = edge_index[0], edge_index[1]
    gs = src // NPG
    assert np.array_equal(dst // NPG, gs), "cross-graph edges unsupported"
    sl = src % NPG
    dl = dst % NPG

    deg = np.bincount(dst, minlength=N).astype(np.float32) + 1.0
    dinv = 1.0 / np.sqrt(deg)

    # unique (g, s, d) with multiplicity, self loops appended; coefficient
    # cnt * dinv[s] * dinv[d] prescaled on host
    gg = np.arange(G, dtype=np.int64).repeat(NPG)
    nn = np.tile(np.arange(NPG, dtype=np.int64), G)
    g_all = np.concatenate([gs, gg])
    s_all = np.concatenate([sl, nn])
    d_all = np.concatenate([dl, nn])
    key = (g_all * NPG + s_all) * NPG + d_all
    uk, cnt = np.unique(key, return_counts=True)
    ud = (uk % NPG).astype(np.int16)
    row = (uk // NPG).astype(np.int64)  # g*NPG + s
    dglob = row // NPG * NPG + uk % NPG
    coef = cnt.astype(np.float32) * dinv[row] * dinv[dglob]
    row_start = np.searchsorted(row, np.arange(N))
    pos = np.arange(len(row)) - row_start[row]
    assert pos.max() < W, f"out-degree overflow: {pos.max() + 1} > {W}"
    EI = np.full((N, W), -1, np.int16)
    EV = np.zeros((N, W), np.float32)
    EI[row, pos] = ud
    EV[row, pos] = coef

    import ml_dtypes
    _bf = ml_dtypes.bfloat16
    rng = np.random.default_rng(12345)
    wts = dict(
        embW=np.ascontiguousarray(np.asarray(inputs["emb_W"], np.float32)).astype(_bf),
        convW=np.ascontiguousarray(np.asarray(inputs["conv_W"], np.float32)[:L]).astype(_bf),
        fconvW=np.ascontiguousarray(np.asarray(inputs["fconv_W"], np.float32)[:L]).astype(_bf),
        gateW=np.ascontiguousarray(np.asarray(inputs["gate_W"], np.float32)).astype(_bf),
    )
    fvec = np.zeros((128, FV_N), np.float32)

    def setv(col, vec):
        fvec[:, col] = vec[0:128]
        fvec[:, col + 1] = vec[128:256]

    setv(FV_EMB_B, np.asarray(inputs["emb_b"], np.float32))
    setv(FV_GATE_B, np.asarray(inputs["gate_b"], np.float32))
    for l in range(L):
        base = FV_L + l * 16
        setv(base + 0, np.asarray(inputs["conv_b"], np.float32)[l])
        setv(base + 2, np.asarray(inputs["norm_w"], np.float32)[l])
        setv(base + 4, np.asarray(inputs["norm_b"], np.float32)[l])
        setv(base + 6, np.asarray(inputs["norm_ms"], np.float32)[l])
        setv(base + 8, np.asarray(inputs["fconv_b"], np.float32)[l])
        setv(base + 10, np.asarray(inputs["fnorm_w"], np.float32)[l])
        setv(base + 12, np.asarray(inputs["fnorm_b"], np.float32)[l])
        setv(base + 14, np.asarray(inputs["fnorm_ms"], np.float32)[l])

    in_maps = []
    for c in range(N_CORES):
        g0, ng = STARTS[c], NGS[c]
        xT = np.zeros((GPC, IN, NP), np.float32)
        ei_c = np.full((GPC, 4, 128, W), -1, np.int16)
        ev_c = np.zeros((GPC, 4, 128, W), np.float32)
        for j in range(GPC):
            if j < ng:
                g = g0 + j
                xg = x[g * NPG:(g + 1) * NPG]
            else:
                xg = rng.standard_normal((NPG, IN)).astype(np.float32)
            xT[j, :, 0:NPG] = xg.T
            if j < ng:
                eig = np.full((NP, W), -1, np.int16)
                evg = np.zeros((NP, W), np.float32)
                eig[0:NPG] = EI[g * NPG:(g + 1) * NPG]
                evg[0:NPG] = EV[g * NPG:(g + 1) * NPG]
                ei_c[j] = eig.reshape(4, 128, W)
                ev_c[j] = evg.reshape(4, 128, W)
        in_maps.append(dict(
            xT=xT.astype(_bf),
            ei=ei_c,
            ev=ev_c.astype(_bf),
            fvec=fvec,
            **wts,
        ))
    return in_maps


_prog_cache = {}


def _get_program():
    if "nc" not in _prog_cache:
        _prog_cache["nc"] = build_program(GPC)
    return _prog_cache["nc"]


def kernel(**inputs):
    in_maps = prep_inputs(inputs)
    nc = _get_program()
    trace = os.environ.get("KERNEL_TRACE", "0") == "1"
    kw = {}
    if trace:
        import antenv
        p = "/opt/trn_rl_repo/antenv"
        if p not in antenv.__path__:
            antenv.__path__.append(p)
        from antenv.axon_hooks import get_axon_ntff_profile_hook, set_axon_ntff_profile_hook
        if get_axon_ntff_profile_hook() is None:
            from trn_agent_boot.trn_boot import _ntff_profile_via_ctypes
            set_axon_ntff_profile_hook(_ntff_profile_via_ctypes("/opt/axon/libaxon_pjrt.so"))
        from concourse import bass_utils as _bu
        _bu.upload_artifacts = lambda tmpdir: "local://" + tmpdir
        base = os.environ.get("KERNEL_TRACE_DIR")
        if base:
            _prog_cache["run_id"] = _prog_cache.get("run_id", 0) + 1
            tdir = os.path.join(base, f"run{_prog_cache['run_id']}")
            os.makedirs(tdir, exist_ok=True)
        else:
            tdir = None
        kw = dict(trace=True, tmpdir=tdir)
    res = run_bass_kernel_spmd(nc, in_maps, core_ids=list(range(N_CORES)), **kw)
    if trace:
        print(f"HW exec time: {res.exec_time_ns} ns")
    out = np.zeros((G, H), np.float32)
    for c in range(N_CORES):
        g0, ng = STARTS[c], NGS[c]
        out[g0:g0 + ng] = res.results[c]["gf"][0:ng]
    return out
